# revision 5
# baseline (speedup 1.0000x reference)
"""Trainium2 kernel for nn_CompLinear3 (vq_codebook).

Strategy (2D shard: 4 token-quarters x 2 out-halves over 8 cores):
- The VQ decode MLP is an affine map of each gathered codebook row, so it is
  precomputed ONCE over the 65536-entry codebook (a 4MB table) on the host;
  the per-block decode collapses to a table gather (numpy, ~0.1s).
- De-standardization is folded into the epilogue:
      y = scale . (x @ Wr^T + (shift/scale) xs + bias/scale) ,  xs = rowsum(x)
  so the raw gathered Wr ships as bf16 with no host rewrite, and the device
  adds the shift/bias terms via a K=2 augmented matmul into the same PSUM.
- Each core gets x[2048 tok] (bf16, natural layout; DMA-transposed on device)
  and its Wr half as int8 [k, o] (global scale folded into the epilogue;
  dequantized to bf16 on the Act engine before the PE). Output is fp16
  [2048 o, 2048 t]. Wire traffic/call: ~320MB vs ~800MB for the naive
  column-parallel plan (the axon tunnel at ~45MB/s is the true bottleneck).
"""
import numpy as np
import ml_dtypes

IN_F = 4096
OUT_F = 4096
TOK = 8192
NCORES = 8
TQ = TOK // 4        # 2048 tokens per quarter
OH = OUT_F // 2      # 2048 out-features per half
NOT = OH // 128      # 16 o-tiles per half
KT = IN_F // 128     # 32 k-tiles

_CACHE = {}


def _build():
    import concourse.bacc as bacc
    import concourse.mybir as mybir
    import concourse.tile as tile

    nc = bacc.Bacc("TRN2", target_bir_lowering=False, debug=False)
    xq = nc.dram_tensor("xq", [TQ, IN_F], mybir.dt.bfloat16, kind="ExternalInput")
    wt = nc.dram_tensor("wt", [IN_F, OH], mybir.dt.int8, kind="ExternalInput")
    aug = nc.dram_tensor("aug", [2, OH], mybir.dt.bfloat16, kind="ExternalInput")
    xs1 = nc.dram_tensor("xs1", [2, TQ], mybir.dt.bfloat16, kind="ExternalInput")
    scl = nc.dram_tensor("scl", [128, NOT], mybir.dt.float32, kind="ExternalInput")
    out = nc.dram_tensor("o", [OH, TQ], mybir.dt.float16, kind="ExternalOutput")

    wtv = wt[:].rearrange("(n p) o -> n p o", p=128)
    with tile.TileContext(nc) as tc:
        with tc.tile_pool(name="cst", bufs=1) as cst, \
             tc.tile_pool(name="w8", bufs=2) as w8p, \
             tc.tile_pool(name="wk", bufs=2) as wkp, \
             tc.tile_pool(name="op", bufs=4) as op, \
             tc.tile_pool(name="ps", bufs=4, space="PSUM") as ps:
            aug_sb = cst.tile([2, OH], mybir.dt.bfloat16)
            nc.sync.dma_start(aug_sb[:], aug[:])
            xs1_sb = cst.tile([2, TQ], mybir.dt.bfloat16)
            nc.sync.dma_start(xs1_sb[:], xs1[:])
            scl_sb = cst.tile([128, NOT], mybir.dt.float32)
            nc.sync.dma_start(scl_sb[:], scl[:])

            # x^T resident in SBUF: 32 k-tiles of [128, 2048]
            xt = []
            for s in range(KT):
                t = cst.tile([128, TQ], mybir.dt.bfloat16, tag=f"xt{s}")
                nc.sync.dma_start_transpose(t[:], xq[:, s * 128:(s + 1) * 128])
                xt.append(t)

            for ot in range(NOT):
                # W^T tiles for this o-tile: int8 [128 k, 128 o] x 32, dequant
                w8 = w8p.tile([128, KT * 128], mybir.dt.int8, tag="w8")
                for s in range(KT):
                    nc.sync.dma_start(
                        w8[:, s * 128:(s + 1) * 128],
                        wtv[s][:, ot * 128:(ot + 1) * 128])
                wk = wkp.tile([128, KT * 128], mybir.dt.bfloat16, tag="wk")
                nc.scalar.copy(wk[:], w8[:])
                for tch in range(4):
                    psum = ps.tile([128, 512], mybir.dt.float32, tag="ps")
                    for s in range(KT):
                        nc.tensor.matmul(
                            psum[:],
                            wk[:, s * 128:(s + 1) * 128],
                            xt[s][:, tch * 512:(tch + 1) * 512],
                            start=(s == 0), stop=False)
                    nc.tensor.matmul(
                        psum[:],
                        aug_sb[:, ot * 128:(ot + 1) * 128],
                        xs1_sb[:, tch * 512:(tch + 1) * 512],
                        start=False, stop=True)
                    o_sb = op.tile([128, 512], mybir.dt.float16, tag="o")
                    nc.vector.tensor_scalar_mul(o_sb[:], psum[:], scl_sb[:, ot:ot + 1])
                    nc.sync.dma_start(
                        out[ot * 128:(ot + 1) * 128, tch * 512:(tch + 1) * 512],
                        o_sb[:])
    nc.compile()
    return nc


def kernel(x, y_in_idx, codebook, W1, b1, W2, b2, scale, shift, bias):
    from concourse.bass_utils import run_bass_kernel_spmd

    x = np.asarray(x, np.float32)
    yi = np.asarray(y_in_idx).astype(np.int64)
    codebook = np.asarray(codebook, np.float32)
    W1 = np.asarray(W1, np.float32); b1 = np.asarray(b1, np.float32)
    W2 = np.asarray(W2, np.float32); b2 = np.asarray(b2, np.float32)
    scale = np.asarray(scale, np.float32); shift = np.asarray(shift, np.float32)
    bias = np.asarray(bias, np.float32)

    # Decode = gather from the once-decoded codebook table (no 1M-row MLP).
    # Quantize the 65536-entry table, then gather int8 (16x less quant work).
    T = np.maximum(codebook @ W1 + b1, 0.0) @ W2 + b2          # [65536, 16]
    ws = float(np.abs(T).max()) / 127.0                         # global int8 scale
    T8 = np.clip(np.rint(T * (1.0 / ws)), -127, 127).astype(np.int8)
    Wt8 = np.ascontiguousarray(T8[yi].reshape(OUT_F, IN_F).T)   # [k, o] int8
    x8 = x.reshape(TOK, IN_F)
    xs = x8.sum(axis=1)                                         # fp32 rowsum
    xb = x8.astype(ml_dtypes.bfloat16)
    aug_full = np.ascontiguousarray(
        np.stack([shift / (scale * ws), bias / (scale * ws)])).astype(ml_dtypes.bfloat16)
    scl_full = np.ascontiguousarray(
        (scale * ws).reshape(2, NOT, 128).transpose(0, 2, 1)).astype(np.float32)

    if "nc" not in _CACHE:
        _CACHE["nc"] = _build()
    nc = _CACHE["nc"]

    ones = np.ones(TQ, np.float32)
    in_maps = []
    for c in range(NCORES):
        q, h = c % 4, c // 4
        in_maps.append({
            "xq": xb[q * TQ:(q + 1) * TQ],
            "wt": np.ascontiguousarray(Wt8[:, h * OH:(h + 1) * OH]),
            "aug": np.ascontiguousarray(aug_full[:, h * OH:(h + 1) * OH]),
            "xs1": np.stack([xs[q * TQ:(q + 1) * TQ], ones]).astype(ml_dtypes.bfloat16),
            "scl": np.ascontiguousarray(scl_full[h]),
        })

    res = None
    for attempt in range(3):
        try:
            res = run_bass_kernel_spmd(nc, in_maps, core_ids=list(range(NCORES)))
            break
        except Exception:
            # transient NRT/axon device hiccups: rebuild once and retry
            if attempt == 2:
                raise
            _CACHE.pop("nc", None)
            _CACHE["nc"] = nc = _build()
    _CACHE["last_exec_ns"] = res.exec_time_ns

    full = np.empty((OUT_F, TOK), np.float16)
    for c in range(NCORES):
        q, h = c % 4, c // 4
        full[h * OH:(h + 1) * OH, q * TQ:(q + 1) * TQ] = res.results[c]["o"]
    return np.ascontiguousarray(full.T).astype(np.float32).reshape(4, 2048, IN_F)


# revision 6
# speedup vs baseline: 1.0551x; 1.0551x over previous
"""Trainium2 kernel for nn_CompLinear3 (vq_codebook).

Strategy (2D shard: 4 token-quarters x 2 out-halves over 8 cores):
- The VQ decode MLP is an affine map of each gathered codebook row, so it is
  precomputed ONCE over the 65536-entry codebook (a 4MB table) on the host;
  the per-block decode collapses to a table gather (numpy, ~0.1s).
- De-standardization is folded into the epilogue:
      y = scale . (x @ Wr^T + (shift/scale) xs + bias/scale) ,  xs = rowsum(x)
  so the raw gathered Wr ships as bf16 with no host rewrite, and the device
  adds the shift/bias terms via a K=2 augmented matmul into the same PSUM.
- Each core gets x[2048 tok] (bf16, natural layout; DMA-transposed on device)
  and its Wr half as int8 [k, o] (global scale folded into the epilogue;
  dequantized to bf16 on the Act engine before the PE). Output is fp16
  [2048 o, 2048 t]. Wire traffic/call: ~320MB vs ~800MB for the naive
  column-parallel plan (the axon tunnel at ~45MB/s is the true bottleneck).
"""
import numpy as np
import ml_dtypes

IN_F = 4096
OUT_F = 4096
TOK = 8192
NCORES = 8
TQ = TOK // 4        # 2048 tokens per quarter
OH = OUT_F // 2      # 2048 out-features per half
NOT = OH // 128      # 16 o-tiles per half
KT = IN_F // 128     # 32 k-tiles

_CACHE = {}


def _build():
    import concourse.bacc as bacc
    import concourse.mybir as mybir
    import concourse.tile as tile

    nc = bacc.Bacc("TRN2", target_bir_lowering=False, debug=False)
    xq = nc.dram_tensor("xq", [IN_F, TQ], mybir.dt.int8, kind="ExternalInput")
    wt = nc.dram_tensor("wt", [IN_F, OH], mybir.dt.int8, kind="ExternalInput")
    aug = nc.dram_tensor("aug", [2, OH], mybir.dt.bfloat16, kind="ExternalInput")
    xs1 = nc.dram_tensor("xs1", [2, TQ], mybir.dt.bfloat16, kind="ExternalInput")
    scl = nc.dram_tensor("scl", [128, NOT], mybir.dt.float32, kind="ExternalInput")
    out = nc.dram_tensor("o", [OH, TQ], mybir.dt.float16, kind="ExternalOutput")

    wtv = wt[:].rearrange("(n p) o -> n p o", p=128)
    xqv = xq[:].rearrange("(n p) t -> n p t", p=128)
    with tile.TileContext(nc) as tc:
        with tc.tile_pool(name="cst", bufs=1) as cst, \
             tc.tile_pool(name="x8", bufs=2) as x8p, \
             tc.tile_pool(name="w8", bufs=2) as w8p, \
             tc.tile_pool(name="wk", bufs=2) as wkp, \
             tc.tile_pool(name="op", bufs=4) as op, \
             tc.tile_pool(name="ps", bufs=4, space="PSUM") as ps:
            aug_sb = cst.tile([2, OH], mybir.dt.bfloat16)
            nc.sync.dma_start(aug_sb[:], aug[:])
            xs1_sb = cst.tile([2, TQ], mybir.dt.bfloat16)
            nc.sync.dma_start(xs1_sb[:], xs1[:])
            scl_sb = cst.tile([128, NOT], mybir.dt.float32)
            nc.sync.dma_start(scl_sb[:], scl[:])

            # x^T resident in SBUF: 32 k-tiles of [128, 2048],
            # shipped int8 (host-transposed), dequantized to bf16 here
            xt = []
            for s in range(KT):
                x8 = x8p.tile([128, TQ], mybir.dt.int8, tag="x8")
                nc.sync.dma_start(x8[:], xqv[s])
                t = cst.tile([128, TQ], mybir.dt.bfloat16, tag=f"xt{s}")
                nc.scalar.copy(t[:], x8[:])
                xt.append(t)

            for ot in range(NOT):
                # W^T tiles for this o-tile: int8 [128 k, 128 o] x 32, dequant
                w8 = w8p.tile([128, KT * 128], mybir.dt.int8, tag="w8")
                for s in range(KT):
                    nc.sync.dma_start(
                        w8[:, s * 128:(s + 1) * 128],
                        wtv[s][:, ot * 128:(ot + 1) * 128])
                wk = wkp.tile([128, KT * 128], mybir.dt.bfloat16, tag="wk")
                nc.scalar.copy(wk[:], w8[:])
                for tch in range(4):
                    psum = ps.tile([128, 512], mybir.dt.float32, tag="ps")
                    for s in range(KT):
                        nc.tensor.matmul(
                            psum[:],
                            wk[:, s * 128:(s + 1) * 128],
                            xt[s][:, tch * 512:(tch + 1) * 512],
                            start=(s == 0), stop=False)
                    nc.tensor.matmul(
                        psum[:],
                        aug_sb[:, ot * 128:(ot + 1) * 128],
                        xs1_sb[:, tch * 512:(tch + 1) * 512],
                        start=False, stop=True)
                    o_sb = op.tile([128, 512], mybir.dt.float16, tag="o")
                    nc.vector.tensor_scalar_mul(o_sb[:], psum[:], scl_sb[:, ot:ot + 1])
                    nc.sync.dma_start(
                        out[ot * 128:(ot + 1) * 128, tch * 512:(tch + 1) * 512],
                        o_sb[:])
    nc.compile()
    return nc


def kernel(x, y_in_idx, codebook, W1, b1, W2, b2, scale, shift, bias):
    from concourse.bass_utils import run_bass_kernel_spmd

    x = np.asarray(x, np.float32)
    yi = np.asarray(y_in_idx).astype(np.int64)
    codebook = np.asarray(codebook, np.float32)
    W1 = np.asarray(W1, np.float32); b1 = np.asarray(b1, np.float32)
    W2 = np.asarray(W2, np.float32); b2 = np.asarray(b2, np.float32)
    scale = np.asarray(scale, np.float32); shift = np.asarray(shift, np.float32)
    bias = np.asarray(bias, np.float32)

    # Decode = gather from the once-decoded codebook table (no 1M-row MLP).
    # Quantize the 65536-entry table, then gather int8 (16x less quant work).
    T = np.maximum(codebook @ W1 + b1, 0.0) @ W2 + b2          # [65536, 16]
    ws = float(np.abs(T).max()) / 127.0                         # global int8 scale
    T8 = np.clip(np.rint(T * (1.0 / ws)), -127, 127).astype(np.int8)
    Wt8 = np.ascontiguousarray(T8[yi].reshape(OUT_F, IN_F).T)   # [k, o] int8
    x2 = x.reshape(TOK, IN_F)
    xs = x2.sum(axis=1)                                         # fp32 rowsum
    sx = np.abs(x2).max(axis=1) / 127.0                         # per-token scale
    sbar = float(sx.mean())
    xq8 = np.ascontiguousarray(np.clip(
        np.rint(x2 * (1.0 / sx)[:, None]), -127, 127).astype(np.int8).T)  # [k, t]
    aug_full = np.ascontiguousarray(
        np.stack([shift / scale, bias / scale])).astype(ml_dtypes.bfloat16)
    scl_full = np.ascontiguousarray(
        (scale * ws * sbar).reshape(2, NOT, 128).transpose(0, 2, 1)).astype(np.float32)
    iws = 1.0 / (ws * sx)                                       # fp32 [TOK]

    if "nc" not in _CACHE:
        _CACHE["nc"] = _build()
    nc = _CACHE["nc"]

    in_maps = []
    for c in range(NCORES):
        q, h = c % 4, c // 4
        tq = slice(q * TQ, (q + 1) * TQ)
        in_maps.append({
            "xq": xq8[:, tq],
            "wt": np.ascontiguousarray(Wt8[:, h * OH:(h + 1) * OH]),
            "aug": np.ascontiguousarray(aug_full[:, h * OH:(h + 1) * OH]),
            "xs1": np.stack([xs[tq] * iws[tq], iws[tq]]).astype(ml_dtypes.bfloat16),
            "scl": np.ascontiguousarray(scl_full[h]),
        })

    res = None
    for attempt in range(3):
        try:
            res = run_bass_kernel_spmd(nc, in_maps, core_ids=list(range(NCORES)))
            break
        except Exception:
            # transient NRT/axon device hiccups: rebuild once and retry
            if attempt == 2:
                raise
            _CACHE.pop("nc", None)
            _CACHE["nc"] = nc = _build()
    _CACHE["last_exec_ns"] = res.exec_time_ns

    full = np.empty((OUT_F, TOK), np.float16)
    for c in range(NCORES):
        q, h = c % 4, c // 4
        full[h * OH:(h + 1) * OH, q * TQ:(q + 1) * TQ] = res.results[c]["o"]
    y = full.T.astype(np.float32)
    y *= (sx * (1.0 / sbar))[:, None]          # per-token int8 scale residual
    return np.ascontiguousarray(y).reshape(4, 2048, IN_F)
